# revision 18
# baseline (speedup 1.0000x reference)
"""BitPackedLinear Trainium2 kernel (8-core SPMD, token-sharded, fp8 DoubleRow).

y = x @ W.T + bias, W = unpack_bits(packed_weight) in {-1,+1}, shapes:
  x [2, 2048, 4096] f32, packed_weight [4096, 512] u8, bias [4096] f32.

Sharding: data-parallel over tokens (4096 tokens -> 512/core). Each core
computes y_c = x_c @ W.T + bias for its token shard against the full
weight; the host just concatenates shards.

Device algorithm per core:
  - W = 2B - 1, so y = 2*(x@B.T) - rowsum(x) + bias. The matmul runs on
    B2 = 2B in {0,2} (exact in fp8 e4m3, pattern 0x40).
  - x is split hi/lo from bf16: xh = e4m3(x), xl = e4m3(x - xh); fp8
    DoubleRow matmuls (2 k-rows per PE cell, 0.5 cyc/row) do the work.
    Only LO_U of the 16 weight-pair tiles get a lo plane; the rest run
    hi-only (pure fp8), trading bounded quantization error for PE time.
  - Contraction tiling: u16-tile u=(blk,b) pairs the two i-tiles
    A=(blk,s=0,b), B=(blk,s=1,b) where i = 2048*blk + 16*m + 8*s + b at
    partition m. The packed bytes transpose as RAW u16 pairs (no dtype
    cast): byteT2[m, blk, o] holds bytes (2m, 2m+1) of pw row o, so ONE
    tensor_scalar (shl 6-b & 0x4040, shr 1 for b=7) unpacks BOTH tiles'
    fp8 weights interleaved per u16 lane. The DR matmul rhs reads them
    with a stride-2 AP; lhsT pairs (hi_A,hi_B) / (lo_A,lo_B) are adjacent
    xT8 slots. No pairing chain, no wraparound copies.
  - x rides 8 big HWDGE DMAs as f32 (one per (blk, token-tile)), is cast
    to bf16 (split across DVE/ACT/gpsimd), PE-transposed in 8-plane PSUM
    batches; ACT extracts hi (e4m3) with stride-2 slot APs, DVE subtracts
    the residual lo. Slab-0 matmul bursts chase the chunk arrivals.
  - s_col[t] = -sum_i(xh+xl) via (-1)-ones DoubleRow matmuls over the
    same pairs; its [1,T] psum row is PE-transposed back to [t,1].
  - Weight prefetch: slab sl+1 bytes DMA + transpose + unpack are emitted
    mid-slab sl (bufs=2 rings).
  - Epilogue fuses (psum + neg_s) + bias on DVE; bias is a stride-0
    broadcast DMA per slab. DMA queues are split: x + y stores on SP
    HWDGE, pk + bias on ACT HWDGE, so descriptor-gen never serializes
    the x stream.
"""
import sys

sys.path.insert(0, "/opt/trn_rl_repo")
from contextlib import ExitStack

import numpy as np

import concourse.tile as tile
from concourse import bacc, mybir
from concourse.bass import ts
from concourse.bass_utils import run_bass_kernel_spmd
from concourse.masks import make_identity

F32 = mybir.dt.float32
F16 = mybir.dt.float16
BF16 = mybir.dt.bfloat16
U8 = mybir.dt.uint8
U16 = mybir.dt.uint16
FP8 = mybir.dt.float8e4
P = 128

N_CORES = 8
B_DIM, S_DIM, I_DIM, O_DIM = 2, 2048, 4096, 4096
T_FULL = B_DIM * S_DIM          # 4096 tokens
T_SHARD = T_FULL // N_CORES     # 512 tokens per core
OUT_NAME = "y"
OUT_SHAPE = (T_SHARD, O_DIM)

DR = mybir.MatmulPerfMode.DoubleRow
SHL = mybir.AluOpType.logical_shift_left
SHR = mybir.AluOpType.logical_shift_right
AND = mybir.AluOpType.bitwise_and
SUB = mybir.AluOpType.subtract
ADD = mybir.AluOpType.add


def build(T=T_SHARD, I=I_DIM, O=O_DIM, O_SLAB=512, LO_U=8, n_cores=N_CORES):
    assert I == 4096 and T % P == 0 and O % O_SLAB == 0
    BLK = 2                 # 2048-wide i-blocks
    CPB = 16                # transpose planes per block (c = 8*s + b)
    NU = 16                 # u16 weight-pair tiles (each = 2 i-tiles)
    assert 8 <= LO_U <= NU
    TT = T // P             # token tiles
    K = I // 8              # packed bytes per weight row
    NSLAB = O // O_SLAB
    OSL_T = O_SLAB // P
    NSLOT = 2 * NU + 2 * LO_U   # xT8 slots: 32 hi + 2*LO_U lo

    nc = bacc.Bacc("TRN2", target_bir_lowering=False, debug=False,
                   num_devices=n_cores)
    x_d = nc.dram_tensor("x", [T, I], F32, kind="ExternalInput").ap()
    pw_d = nc.dram_tensor("pw", [O, K], U8, kind="ExternalInput").ap()
    bias_d = nc.dram_tensor("bias", [O], F32, kind="ExternalInput").ap()
    y_d = nc.dram_tensor(OUT_NAME, [T, O], F32, kind="ExternalOutput").ap()

    def lo_range(blk):
        return range(8 * blk, min(8 * blk + 8, LO_U))

    with tile.TileContext(nc) as tc:
        with ExitStack() as ctx:
            const = ctx.enter_context(tc.tile_pool(name="const", bufs=1))
            persist = ctx.enter_context(tc.tile_pool(name="persist", bufs=1))

            ident_bf = const.tile([P, P], BF16)
            make_identity(nc, ident_bf[:])
            ident_f16 = const.tile([P, P], F16)
            make_identity(nc, ident_f16[:])
            ident_f = const.tile([P, P], F32)
            make_identity(nc, ident_f[:])
            nones8 = const.tile([P, 2, P], FP8)
            nc.vector.memset(nones8[:], -1.0)

            xT8 = persist.tile([P, NSLOT, T], FP8)
            neg_s = persist.tile([P, TT], F32)
            pw_ap = pw_d.rearrange("(ot p) k -> p ot k", p=P)
            bias_bc = bias_d.rearrange("(b o) -> b o", b=1)

            pk_pool = ctx.enter_context(tc.tile_pool(name="pk", bufs=2))
            byteT_pool = ctx.enter_context(tc.tile_pool(name="byteT", bufs=2))
            x32_pool = ctx.enter_context(tc.tile_pool(name="x32", bufs=5))
            xn_pool = ctx.enter_context(tc.tile_pool(name="xn", bufs=BLK * TT))
            wt_pool = ctx.enter_context(tc.tile_pool(name="wt", bufs=2))
            bbc_pool = ctx.enter_context(tc.tile_pool(name="bbc", bufs=2))
            y_pool = ctx.enter_context(tc.tile_pool(name="ysb", bufs=4))
            stage_ps = ctx.enter_context(
                tc.tile_pool(name="stage_ps", bufs=3, space="PSUM")
            )
            ps_mm = ctx.enter_context(
                tc.tile_pool(name="ps_mm", bufs=5, space="PSUM")
            )

            def pk_dma(sl, eng=None):
                # one DMA per o-slab: [o-part, otl, bytes]; slabs 0/1 ride
                # the SP queue ahead of x, later slabs the ACT queue
                pk = pk_pool.tile([P, OSL_T, K], U8)
                (eng or nc.scalar).dma_start(
                    pk[:], pw_ap[:, sl * OSL_T:(sl + 1) * OSL_T, :]
                )
                return pk

            def bias_dma(sl):
                bbc = bbc_pool.tile([P, O_SLAB], F32)
                nc.scalar.dma_start(
                    bbc[:],
                    bias_bc[:, ts(sl, O_SLAB)].partition_broadcast(P),
                )
                return bbc

            def byte_slab(pk):
                """Raw u16-pair transpose of packed bytes: byteT2[m, blk, o]
                = bytes (2m, 2m+1) of block blk of pw row o."""
                byteT2 = byteT_pool.tile([P, BLK, O_SLAB], U16)
                for otl in range(OSL_T):
                    # raw byte-pairs transposed as f16 bits (pure pass-through)
                    psb = stage_ps.tile([P, BLK, P], F16, tag="stg")
                    pk16 = pk[:, otl, :].bitcast(F16)
                    for blk in range(BLK):
                        nc.tensor.transpose(
                            psb[:, blk, :], pk16[:, ts(blk, P)], ident_f16[:]
                        )
                    nc.scalar.copy(
                        out=byteT2[:, :, ts(otl, P)], in_=psb[:].bitcast(U16)
                    )
                return byteT2

            def unpack_tiles(byteT2, wt, urange):
                for u in urange:
                    blk, b = divmod(u, 8)
                    sh, op = (6 - b, SHL) if b < 7 else (1, SHR)
                    nc.vector.tensor_scalar(
                        out=wt[:, u, :], in0=byteT2[:, blk, :], scalar1=sh,
                        scalar2=0x4040, op0=op, op1=AND,
                    )

            def unpack_slab(byteT2):
                """wt[:, u, :] u16 = fp8 pair (w_A, w_B) of u16-tile u,
                interleaved per output o (A low byte, B high byte)."""
                wt = wt_pool.tile([P, NU, O_SLAB], U16)
                unpack_tiles(byteT2, wt, range(NU))
                return wt

            def mm_rhs(wt8, u):
                # stride-2 de-interleave: slot 0 = w_A (even fp8), 1 = w_B
                return wt8[:, u, :].rearrange("p (o s) -> p s o", s=2)

            def mm_lhs_hi(u, t0, tn):
                return xT8[:, 2 * u:2 * u + 2, t0:t0 + tn]

            def mm_lhs_lo(h, t0, tn):
                base = 2 * NU + 2 * h
                return xT8[:, base:base + 2, t0:t0 + tn]

            # --- upfront DMA issue on SP: x chunk 0 first (it gates the
            # whole transpose pipeline), pk0 next, then the rest of x ---
            def x_dma(blk, tt):
                x32 = x32_pool.tile([P, P, CPB], F32)
                nc.sync.dma_start(
                    x32[:],
                    x_d[ts(tt, P), ts(blk, 2048)].rearrange(
                        "p (m c) -> p m c", c=CPB
                    ),
                )
                x32s[blk, tt] = x32
            x32s = {}
            x_dma(0, 0)
            pk_cur = pk_dma(0, nc.sync)
            for blk in range(BLK):
                for tt in range(TT):
                    if (blk, tt) != (0, 0):
                        x_dma(blk, tt)

            # slab-0 weights: transpose + unpack while x streams in;
            # pk1 gen sits behind the byteT2 copies on ACT SEQ so its
            # transfer never delays the x stream. Only the blk0 tiles are
            # unpacked ahead of the casts - the first burst needs just those,
            # and DVE must be free when chunk0 lands.
            byteT2_0 = byte_slab(pk_cur)
            wt_cur = wt_pool.tile([P, NU, O_SLAB], U16)
            unpack_tiles(byteT2_0, wt_cur, range(8))
            pk_next = pk_dma(1)

            pss = [
                ps_mm.tile([P, O_SLAB], F32, name=f"ps{t}", tag="ps_y")
                for t in range(TT)
            ]

            def mm_burst(wt8, tt, blk):  # noqa: called with *(tt, blk)
                # hi MMs for this block's 8 u16-tiles, then lo MMs;
                # start on the very first MM of (tt), stop on the last.
                for ui, u in enumerate(range(8 * blk, 8 * blk + 8)):
                    nc.tensor.matmul(
                        pss[tt][:], mm_lhs_hi(u, tt * P, P), mm_rhs(wt8, u),
                        start=(blk == 0 and ui == 0), stop=False,
                        perf_mode=DR,
                    )
                los = list(lo_range(blk))
                for li, h in enumerate(los):
                    last = (blk == BLK - 1) and (li == len(los) - 1)
                    nc.tensor.matmul(
                        pss[tt][:], mm_lhs_lo(h, tt * P, P), mm_rhs(wt8, h),
                        start=False, stop=last, perf_mode=DR,
                    )

            def neg_mms(tt):
                # -rowsum for one token tile as ap_size-1 DR matmuls into a
                # [128,1] psum column, copied straight into neg_s[:, tt]
                psn = stage_ps.tile([P, 1], F32, tag="stg")
                for u in range(NU):
                    nc.tensor.matmul(
                        psn[:], mm_lhs_hi(u, tt * P, P), nones8[:, :, 0:1],
                        start=(u == 0), stop=False, perf_mode=DR,
                    )
                for h in range(LO_U):
                    nc.tensor.matmul(
                        psn[:], mm_lhs_lo(h, tt * P, P), nones8[:, :, 0:1],
                        start=False, stop=(h == LO_U - 1), perf_mode=DR,
                    )
                nc.vector.tensor_copy(out=neg_s[:, tt:tt + 1], in_=psn[:])

            def epilogue(sl, tsub, ps_y, bbc):
                y_sb = y_pool.tile([P, O_SLAB], F32)
                nc.vector.scalar_tensor_tensor(
                    out=y_sb[:], in0=ps_y[:],
                    scalar=neg_s[:, tsub:tsub + 1],
                    in1=bbc[:], op0=ADD, op1=ADD,
                )
                nc.sync.dma_start(
                    y_d[ts(tsub, P), ts(sl, O_SLAB)], y_sb[:]
                )

            # --- slab 0: chunk-chasing pipeline ---
            wt8 = wt_cur[:].bitcast(FP8)
            chunks = [(blk, tt) for blk in range(BLK) for tt in range(TT)]
            # blk0 chunks (which carry the lo planes) go through bf16 casts
            # split into per-half tiles so each transpose batch depends on
            # exactly one cast; blk1 chunks transpose raw f32 (2 cyc/row,
            # but their bursts are hi-only so the PE has the slack) with no
            # cast at all.
            xns = {}
            for ci, (blk, tt) in enumerate(chunks):
                if blk != 0:
                    continue
                xnA = xn_pool.tile([P, P, 8], BF16, tag="xnA")
                xnB = xn_pool.tile([P, P, 8], BF16, tag="xnB")
                if ci == 0:
                    e0, e1 = nc.vector.tensor_copy, nc.scalar.copy
                else:
                    e0 = e1 = nc.gpsimd.tensor_copy
                e0(out=xnA[:], in_=x32s[blk, tt][:, :, 0:8])
                e1(out=xnB[:], in_=x32s[blk, tt][:, :, 8:16])
                xns[blk, tt] = (xnA, xnB)
            unpack_tiles(byteT2_0, wt_cur, range(8, NU))
            wt_next = None
            pending = None
            for ci, (blk, tt) in enumerate(chunks):
                if blk == 0 and ci == 0:
                    # chunk0 rides finer 4-plane batches: its chain is the
                    # kernel's critical-path prologue
                    for q in range(4):
                        ps = stage_ps.tile([P, 4, P], BF16, tag="stg")
                        xnh = xns[blk, tt][q // 2]
                        for j in range(4):
                            nc.tensor.transpose(
                                ps[:, j, :], xnh[:, :, 4 * (q % 2) + j],
                                ident_bf[:]
                            )
                        if q < 2:
                            hi0 = 8 * q
                            lo0 = 2 * NU + 8 * q
                        else:
                            hi0 = 8 * (q - 2) + 1
                            lo0 = 2 * NU + 8 * (q - 2) + 1
                        nc.scalar.copy(
                            out=xT8[:, hi0:hi0 + 8:2, ts(tt, P)], in_=ps[:]
                        )
                        nc.vector.tensor_tensor(
                            out=xT8[:, lo0:lo0 + 7:2, ts(tt, P)],
                            in0=ps[:],
                            in1=xT8[:, hi0:hi0 + 8:2, ts(tt, P)],
                            op=SUB,
                        )
                elif blk == 0:
                    # two 8-plane bf16 batches; hb=0 -> A planes (c=0..7),
                    # hb=1 -> B planes (c=8..15)
                    for hb in range(2):
                        ps = stage_ps.tile([P, 8, P], BF16, tag="stg")
                        xnh = xns[blk, tt][hb]
                        for j in range(8):
                            nc.tensor.transpose(
                                ps[:, j, :], xnh[:, :, j], ident_bf[:]
                            )
                        hi0 = 16 * blk + hb
                        nc.scalar.copy(
                            out=xT8[:, hi0:hi0 + 16:2, ts(tt, P)], in_=ps[:]
                        )
                        cnt = len(lo_range(blk))
                        if cnt:
                            lo0 = 2 * NU + 16 * blk + hb
                            nc.vector.tensor_tensor(
                                out=xT8[:, lo0:lo0 + 2 * cnt - 1:2,
                                        ts(tt, P)],
                                in0=ps[:, 0:cnt, :],
                                in1=xT8[:, hi0:hi0 + 2 * cnt:2, ts(tt, P)],
                                op=SUB,
                            )
                else:
                    # four 4-plane f32 batches, cast-free (blk1 is hi-only);
                    # extracts alternate ACT/DVE to halve the chain latency
                    for q in range(4):
                        ps = stage_ps.tile([P, 4, P], F32, tag="stg")
                        for j in range(4):
                            nc.tensor.transpose(
                                ps[:, j, :],
                                x32s[blk, tt][:, :, 4 * q + j], ident_f[:]
                            )
                        if q < 2:
                            hi0 = 16 * blk + 8 * q
                        else:
                            hi0 = 16 * blk + 8 * (q - 2) + 1
                        ex = nc.scalar.copy if q % 2 == 0 \
                            else nc.vector.tensor_copy
                        ex(
                            out=xT8[:, hi0:hi0 + 8:2, ts(tt, P)], in_=ps[:]
                        )
                # distance-1 pipeline: burst for chunk ci-1 lands after
                # chunk ci's transposes, hiding the extract latency
                if pending is not None:
                    ptt, pblk = pending
                    mm_burst(wt8, ptt, pblk)
                    if pblk == BLK - 1:
                        neg_mms(ptt)
                    pci = ci - 1
                    if pci == 2:
                        # slab-1 weights early: byte transposes/unpack fill
                        # ACT/DVE slack in the blk0 phase
                        wt_next = unpack_slab(byte_slab(pk_next))
                        pk_next = pk_dma(2) if NSLAB > 2 else None
                    if pci == 5:
                        bias_cur = bias_dma(0)
                pending = (tt, blk)
            ptt, pblk = pending
            mm_burst(wt8, ptt, pblk)
            neg_mms(ptt)
            for tsub in range(TT):
                epilogue(0, tsub, pss[tsub], bias_cur)
            wt_cur = wt_next

            # --- steady slabs (pk for slab sl+1 was issued at slab sl-1) ---
            for sl in range(1, NSLAB):
                wt8 = wt_cur[:].bitcast(FP8)
                bbc = bias_dma(sl)
                pss = [
                    ps_mm.tile([P, O_SLAB], F32, name=f"ps{t}", tag="ps_y")
                    for t in range(TT)
                ]
                for tsub in range(TT):
                    if sl == NSLAB - 1 and tsub == TT - 1:
                        # final tile rides two o-half psum groups with
                        # staggered stops: half the epilogue + store overlaps
                        # the other half's matmuls, trimming the drain tail
                        for half in range(2):
                            ph = ps_mm.tile(
                                [P, O_SLAB // 2], F32, name=f"psh{half}",
                                tag="ps_y",
                            )
                            o0 = half * (O_SLAB // 2)

                            def rhs_half(u):
                                return wt8[:, u, 2 * o0:2 * o0 + O_SLAB] \
                                    .rearrange("p (o s) -> p s o", s=2)

                            for u in range(NU):
                                nc.tensor.matmul(
                                    ph[:], mm_lhs_hi(u, tsub * P, P),
                                    rhs_half(u),
                                    start=(u == 0), stop=False, perf_mode=DR,
                                )
                            for h in range(LO_U):
                                nc.tensor.matmul(
                                    ph[:], mm_lhs_lo(h, tsub * P, P),
                                    rhs_half(h),
                                    start=False, stop=(h == LO_U - 1),
                                    perf_mode=DR,
                                )
                            y_sb = y_pool.tile([P, O_SLAB // 2], F32)
                            nc.vector.scalar_tensor_tensor(
                                out=y_sb[:], in0=ph[:],
                                scalar=neg_s[:, tsub:tsub + 1],
                                in1=bbc[:, o0:o0 + O_SLAB // 2],
                                op0=ADD, op1=ADD,
                            )
                            nc.sync.dma_start(
                                y_d[ts(tsub, P),
                                    sl * O_SLAB + o0:
                                    sl * O_SLAB + o0 + O_SLAB // 2],
                                y_sb[:],
                            )
                        continue
                    for u in range(NU):
                        nc.tensor.matmul(
                            pss[tsub][:], mm_lhs_hi(u, tsub * P, P),
                            mm_rhs(wt8, u),
                            start=(u == 0), stop=False, perf_mode=DR,
                        )
                    for h in range(LO_U):
                        nc.tensor.matmul(
                            pss[tsub][:], mm_lhs_lo(h, tsub * P, P),
                            mm_rhs(wt8, h),
                            start=False, stop=(h == LO_U - 1), perf_mode=DR,
                        )
                    epilogue(sl, tsub, pss[tsub], bbc)
                    if tsub == 1 and pk_next is not None:
                        wt_cur = unpack_slab(byte_slab(pk_next))
                        pk_next = pk_dma(sl + 2) if sl + 2 < NSLAB else None

    nc.compile()
    return nc


_NC = None


def _get_nc():
    global _NC
    if _NC is None:
        _NC = build()
    return _NC


def run(x, packed_weight, bias, trace=False):
    x = np.ascontiguousarray(np.asarray(x, dtype=np.float32))
    pw = np.ascontiguousarray(np.asarray(packed_weight, dtype=np.uint8))
    bias = np.ascontiguousarray(np.asarray(bias, dtype=np.float32))
    assert x.shape == (B_DIM, S_DIM, I_DIM)
    assert pw.shape == (O_DIM, I_DIM // 8)
    assert bias.shape == (O_DIM,)

    nc = _get_nc()
    xs = x.reshape(T_FULL, I_DIM)
    in_maps = [
        {
            "x": np.ascontiguousarray(xs[c * T_SHARD:(c + 1) * T_SHARD]),
            "pw": pw,
            "bias": bias,
        }
        for c in range(N_CORES)
    ]
    res = run_bass_kernel_spmd(nc, in_maps, list(range(N_CORES)), trace=trace)
    y = np.concatenate(
        [res.results[c][OUT_NAME] for c in range(N_CORES)], axis=0
    )
    return y.reshape(B_DIM, S_DIM, O_DIM), res


def kernel(x, packed_weight, bias):
    y, _ = run(x, packed_weight, bias, trace=False)
    return y


# revision 20
# speedup vs baseline: 1.0173x; 1.0173x over previous
"""BitPackedLinear Trainium2 kernel (8-core SPMD, token-sharded, fp8 DoubleRow).

y = x @ W.T + bias, W = unpack_bits(packed_weight) in {-1,+1}, shapes:
  x [2, 2048, 4096] f32, packed_weight [4096, 512] u8, bias [4096] f32.

Sharding: data-parallel over tokens (4096 tokens -> 512/core). Each core
computes y_c = x_c @ W.T + bias for its token shard against the full
weight; the host just concatenates shards.

Device algorithm per core:
  - W = 2B - 1, so y = 2*(x@B.T) - rowsum(x) + bias. The matmul runs on
    B2 = 2B in {0,2} (exact in fp8 e4m3, pattern 0x40).
  - x is split hi/lo from bf16: xh = e4m3(x), xl = e4m3(x - xh); fp8
    DoubleRow matmuls (2 k-rows per PE cell, 0.5 cyc/row) do the work.
    Only LO_U of the 16 weight-pair tiles get a lo plane; the rest run
    hi-only (pure fp8), trading bounded quantization error for PE time.
  - Contraction tiling: u16-tile u=(blk,b) pairs the two i-tiles
    A=(blk,s=0,b), B=(blk,s=1,b) where i = 2048*blk + 16*m + 8*s + b at
    partition m. The packed bytes transpose as RAW u16 pairs (no dtype
    cast): byteT2[m, blk, o] holds bytes (2m, 2m+1) of pw row o, so ONE
    tensor_scalar (shl 6-b & 0x4040, shr 1 for b=7) unpacks BOTH tiles'
    fp8 weights interleaved per u16 lane. The DR matmul rhs reads them
    with a stride-2 AP; lhsT pairs (hi_A,hi_B) / (lo_A,lo_B) are adjacent
    xT8 slots. No pairing chain, no wraparound copies.
  - x rides 8 big HWDGE DMAs as f32 (one per (blk, token-tile)), is cast
    to bf16 (split across DVE/ACT/gpsimd), PE-transposed in 8-plane PSUM
    batches; ACT extracts hi (e4m3) with stride-2 slot APs, DVE subtracts
    the residual lo. Slab-0 matmul bursts chase the chunk arrivals.
  - s_col[t] = -sum_i(xh+xl) via (-1)-ones DoubleRow matmuls over the
    same pairs; its [1,T] psum row is PE-transposed back to [t,1].
  - Weight prefetch: slab sl+1 bytes DMA + transpose + unpack are emitted
    mid-slab sl (bufs=2 rings).
  - Epilogue fuses (psum + neg_s) + bias on DVE; bias is a stride-0
    broadcast DMA per slab. DMA queues are split: x + y stores on SP
    HWDGE, pk + bias on ACT HWDGE, so descriptor-gen never serializes
    the x stream.
"""
import sys

sys.path.insert(0, "/opt/trn_rl_repo")
from contextlib import ExitStack

import numpy as np

import concourse.tile as tile
from concourse import bacc, mybir
from concourse.bass import ts
from concourse.bass_utils import run_bass_kernel_spmd
from concourse.masks import make_identity

F32 = mybir.dt.float32
F16 = mybir.dt.float16
BF16 = mybir.dt.bfloat16
U8 = mybir.dt.uint8
U16 = mybir.dt.uint16
FP8 = mybir.dt.float8e4
P = 128

N_CORES = 8
B_DIM, S_DIM, I_DIM, O_DIM = 2, 2048, 4096, 4096
T_FULL = B_DIM * S_DIM          # 4096 tokens
T_SHARD = T_FULL // N_CORES     # 512 tokens per core
OUT_NAME = "y"
OUT_SHAPE = (T_SHARD, O_DIM)

DR = mybir.MatmulPerfMode.DoubleRow
SHL = mybir.AluOpType.logical_shift_left
SHR = mybir.AluOpType.logical_shift_right
AND = mybir.AluOpType.bitwise_and
SUB = mybir.AluOpType.subtract
ADD = mybir.AluOpType.add


def build(T=T_SHARD, I=I_DIM, O=O_DIM, O_SLAB=512, LO_U=8, n_cores=N_CORES):
    assert I == 4096 and T % P == 0 and O % O_SLAB == 0
    BLK = 2                 # 2048-wide i-blocks
    CPB = 16                # transpose planes per block (c = 8*s + b)
    NU = 16                 # u16 weight-pair tiles (each = 2 i-tiles)
    assert 8 <= LO_U <= NU
    TT = T // P             # token tiles
    K = I // 8              # packed bytes per weight row
    NSLAB = O // O_SLAB
    OSL_T = O_SLAB // P
    NSLOT = 2 * NU + 2 * LO_U   # xT8 slots: 32 hi + 2*LO_U lo

    nc = bacc.Bacc("TRN2", target_bir_lowering=False, debug=False,
                   num_devices=n_cores)
    x_d = nc.dram_tensor("x", [T, I], F32, kind="ExternalInput").ap()
    pw_d = nc.dram_tensor("pw", [O, K], U8, kind="ExternalInput").ap()
    bias_d = nc.dram_tensor("bias", [O], F32, kind="ExternalInput").ap()
    y_d = nc.dram_tensor(OUT_NAME, [T, O], F32, kind="ExternalOutput").ap()

    def lo_range(blk):
        return range(8 * blk, min(8 * blk + 8, LO_U))

    with tile.TileContext(nc) as tc:
        with ExitStack() as ctx:
            const = ctx.enter_context(tc.tile_pool(name="const", bufs=1))
            persist = ctx.enter_context(tc.tile_pool(name="persist", bufs=1))

            ident_bf = const.tile([P, P], BF16)
            make_identity(nc, ident_bf[:])
            ident_f16 = const.tile([P, P], F16)
            make_identity(nc, ident_f16[:])
            ident_f = const.tile([P, P], F32)
            make_identity(nc, ident_f[:])
            nones8 = const.tile([P, 2, P], FP8)
            nc.vector.memset(nones8[:], -1.0)

            xT8 = persist.tile([P, NSLOT, T], FP8)
            neg_s = persist.tile([P, TT], F32)
            pw_ap = pw_d.rearrange("(ot p) k -> p ot k", p=P)
            bias_bc = bias_d.rearrange("(b o) -> b o", b=1)

            pk_pool = ctx.enter_context(tc.tile_pool(name="pk", bufs=2))
            byteT_pool = ctx.enter_context(tc.tile_pool(name="byteT", bufs=2))
            x32_pool = ctx.enter_context(tc.tile_pool(name="x32", bufs=5))
            xn_pool = ctx.enter_context(tc.tile_pool(name="xn", bufs=BLK * TT))
            wt_pool = ctx.enter_context(tc.tile_pool(name="wt", bufs=2))
            bbc_pool = ctx.enter_context(tc.tile_pool(name="bbc", bufs=2))
            y_pool = ctx.enter_context(tc.tile_pool(name="ysb", bufs=4))
            stage_ps = ctx.enter_context(
                tc.tile_pool(name="stage_ps", bufs=4, space="PSUM")
            )
            ps_mm = ctx.enter_context(
                tc.tile_pool(name="ps_mm", bufs=4, space="PSUM")
            )

            def pk_dma(sl, eng=None):
                # one DMA per o-slab: [o-part, otl, bytes]; slabs 0/1 ride
                # the SP queue ahead of x, later slabs the ACT queue
                pk = pk_pool.tile([P, OSL_T, K], U8)
                (eng or nc.scalar).dma_start(
                    pk[:], pw_ap[:, sl * OSL_T:(sl + 1) * OSL_T, :]
                )
                return pk

            def bias_dma(sl):
                bbc = bbc_pool.tile([P, O_SLAB], F32)
                nc.scalar.dma_start(
                    bbc[:],
                    bias_bc[:, ts(sl, O_SLAB)].partition_broadcast(P),
                )
                return bbc

            def byte_slab(pk):
                """Raw u16-pair transpose of packed bytes: byteT2[m, blk, o]
                = bytes (2m, 2m+1) of block blk of pw row o."""
                byteT2 = byteT_pool.tile([P, BLK, O_SLAB], U16)
                for otl in range(OSL_T):
                    # raw byte-pairs transposed as f16 bits (pure pass-through)
                    psb = stage_ps.tile([P, BLK, P], F16, tag="stg")
                    pk16 = pk[:, otl, :].bitcast(F16)
                    for blk in range(BLK):
                        nc.tensor.transpose(
                            psb[:, blk, :], pk16[:, ts(blk, P)], ident_f16[:]
                        )
                    nc.scalar.copy(
                        out=byteT2[:, :, ts(otl, P)], in_=psb[:].bitcast(U16)
                    )
                return byteT2

            def unpack_tiles(byteT2, wt, urange):
                for u in urange:
                    blk, b = divmod(u, 8)
                    sh, op = (6 - b, SHL) if b < 7 else (1, SHR)
                    nc.vector.tensor_scalar(
                        out=wt[:, u, :], in0=byteT2[:, blk, :], scalar1=sh,
                        scalar2=0x4040, op0=op, op1=AND,
                    )

            def unpack_slab(byteT2):
                """wt[:, u, :] u16 = fp8 pair (w_A, w_B) of u16-tile u,
                interleaved per output o (A low byte, B high byte)."""
                wt = wt_pool.tile([P, NU, O_SLAB], U16)
                unpack_tiles(byteT2, wt, range(NU))
                return wt

            def mm_rhs(wt8, u):
                # stride-2 de-interleave: slot 0 = w_A (even fp8), 1 = w_B
                return wt8[:, u, :].rearrange("p (o s) -> p s o", s=2)

            def mm_lhs_hi(u, t0, tn):
                return xT8[:, 2 * u:2 * u + 2, t0:t0 + tn]

            def mm_lhs_lo(h, t0, tn):
                base = 2 * NU + 2 * h
                return xT8[:, base:base + 2, t0:t0 + tn]

            # --- upfront DMA issue on SP: x chunk 0 first (it gates the
            # whole transpose pipeline), pk0 next, then the rest of x ---
            def x_dma(blk, tt):
                x32 = x32_pool.tile([P, P, CPB], F32)
                nc.sync.dma_start(
                    x32[:],
                    x_d[ts(tt, P), ts(blk, 2048)].rearrange(
                        "p (m c) -> p m c", c=CPB
                    ),
                )
                x32s[blk, tt] = x32
            x32s = {}
            x_dma(0, 0)
            pk_cur = pk_dma(0, nc.sync)
            for blk in range(BLK):
                for tt in range(TT):
                    if (blk, tt) != (0, 0):
                        x_dma(blk, tt)

            # slab-0 weights: transpose + unpack while x streams in;
            # pk1 gen sits behind the byteT2 copies on ACT SEQ so its
            # transfer never delays the x stream. Only the blk0 tiles are
            # unpacked ahead of the casts - the first burst needs just those,
            # and DVE must be free when chunk0 lands.
            byteT2_0 = byte_slab(pk_cur)
            wt_cur = wt_pool.tile([P, NU, O_SLAB], U16)
            unpack_tiles(byteT2_0, wt_cur, range(8))
            pk_next = pk_dma(1)

            pss = [
                ps_mm.tile([P, O_SLAB], F32, name=f"ps{t}", tag="ps_y")
                for t in range(TT)
            ]

            def mm_burst(wt8, tt, blk):  # noqa: called with *(tt, blk)
                # hi MMs for this block's 8 u16-tiles, then lo MMs;
                # start on the very first MM of (tt), stop on the last.
                for ui, u in enumerate(range(8 * blk, 8 * blk + 8)):
                    nc.tensor.matmul(
                        pss[tt][:], mm_lhs_hi(u, tt * P, P), mm_rhs(wt8, u),
                        start=(blk == 0 and ui == 0), stop=False,
                        perf_mode=DR,
                    )
                los = list(lo_range(blk))
                for li, h in enumerate(los):
                    last = (blk == BLK - 1) and (li == len(los) - 1)
                    nc.tensor.matmul(
                        pss[tt][:], mm_lhs_lo(h, tt * P, P), mm_rhs(wt8, h),
                        start=False, stop=last, perf_mode=DR,
                    )

            def neg_mms(tt):
                # -rowsum for one token tile as ap_size-1 DR matmuls into a
                # [128,1] psum column, copied straight into neg_s[:, tt]
                psn = stage_ps.tile([P, 1], F32, tag="stg")
                for u in range(NU):
                    nc.tensor.matmul(
                        psn[:], mm_lhs_hi(u, tt * P, P), nones8[:, :, 0:1],
                        start=(u == 0), stop=False, perf_mode=DR,
                    )
                for h in range(LO_U):
                    nc.tensor.matmul(
                        psn[:], mm_lhs_lo(h, tt * P, P), nones8[:, :, 0:1],
                        start=False, stop=(h == LO_U - 1), perf_mode=DR,
                    )
                nc.vector.tensor_copy(out=neg_s[:, tt:tt + 1], in_=psn[:])

            def epilogue(sl, tsub, ps_y, bbc):
                y_sb = y_pool.tile([P, O_SLAB], F32)
                nc.vector.scalar_tensor_tensor(
                    out=y_sb[:], in0=ps_y[:],
                    scalar=neg_s[:, tsub:tsub + 1],
                    in1=bbc[:], op0=ADD, op1=ADD,
                )
                nc.sync.dma_start(
                    y_d[ts(tsub, P), ts(sl, O_SLAB)], y_sb[:]
                )

            # --- slab 0: chunk-chasing pipeline ---
            wt8 = wt_cur[:].bitcast(FP8)
            chunks = [(blk, tt) for blk in range(BLK) for tt in range(TT)]
            # blk0 chunks (which carry the lo planes) go through bf16 casts
            # split into per-half tiles so each transpose batch depends on
            # exactly one cast; blk1 chunks transpose raw f32 (2 cyc/row,
            # but their bursts are hi-only so the PE has the slack) with no
            # cast at all.
            xns = {}
            for ci, (blk, tt) in enumerate(chunks):
                if blk != 0:
                    continue
                xnA = xn_pool.tile([P, P, 8], BF16, tag="xnA")
                xnB = xn_pool.tile([P, P, 8], BF16, tag="xnB")
                if ci == 0:
                    e0, e1 = nc.vector.tensor_copy, nc.scalar.copy
                else:
                    e0 = e1 = nc.gpsimd.tensor_copy
                e0(out=xnA[:], in_=x32s[blk, tt][:, :, 0:8])
                e1(out=xnB[:], in_=x32s[blk, tt][:, :, 8:16])
                xns[blk, tt] = (xnA, xnB)
            unpack_tiles(byteT2_0, wt_cur, range(8, NU))
            wt_next = None
            pending = None
            for ci, (blk, tt) in enumerate(chunks):
                if blk == 0 and ci == 0:
                    # chunk0 rides finer 4-plane batches: its chain is the
                    # kernel's critical-path prologue
                    for q in range(4):
                        ps = stage_ps.tile([P, 4, P], BF16, tag="stg")
                        xnh = xns[blk, tt][q // 2]
                        for j in range(4):
                            nc.tensor.transpose(
                                ps[:, j, :], xnh[:, :, 4 * (q % 2) + j],
                                ident_bf[:]
                            )
                        if q < 2:
                            hi0 = 8 * q
                            lo0 = 2 * NU + 8 * q
                        else:
                            hi0 = 8 * (q - 2) + 1
                            lo0 = 2 * NU + 8 * (q - 2) + 1
                        nc.scalar.copy(
                            out=xT8[:, hi0:hi0 + 8:2, ts(tt, P)], in_=ps[:]
                        )
                        nc.vector.tensor_tensor(
                            out=xT8[:, lo0:lo0 + 7:2, ts(tt, P)],
                            in0=ps[:],
                            in1=xT8[:, hi0:hi0 + 8:2, ts(tt, P)],
                            op=SUB,
                        )
                elif blk == 0:
                    # two 8-plane bf16 batches; hb=0 -> A planes (c=0..7),
                    # hb=1 -> B planes (c=8..15)
                    for hb in range(2):
                        ps = stage_ps.tile([P, 8, P], BF16, tag="stg")
                        xnh = xns[blk, tt][hb]
                        for j in range(8):
                            nc.tensor.transpose(
                                ps[:, j, :], xnh[:, :, j], ident_bf[:]
                            )
                        hi0 = 16 * blk + hb
                        nc.scalar.copy(
                            out=xT8[:, hi0:hi0 + 16:2, ts(tt, P)], in_=ps[:]
                        )
                        cnt = len(lo_range(blk))
                        if cnt:
                            lo0 = 2 * NU + 16 * blk + hb
                            nc.vector.tensor_tensor(
                                out=xT8[:, lo0:lo0 + 2 * cnt - 1:2,
                                        ts(tt, P)],
                                in0=ps[:, 0:cnt, :],
                                in1=xT8[:, hi0:hi0 + 2 * cnt:2, ts(tt, P)],
                                op=SUB,
                            )
                else:
                    # four 4-plane f32 batches, cast-free (blk1 is hi-only)
                    for q in range(4):
                        ps = stage_ps.tile([P, 4, P], F32, tag="stg")
                        for j in range(4):
                            nc.tensor.transpose(
                                ps[:, j, :],
                                x32s[blk, tt][:, :, 4 * q + j], ident_f[:]
                            )
                        if q < 2:
                            hi0 = 16 * blk + 8 * q
                        else:
                            hi0 = 16 * blk + 8 * (q - 2) + 1
                        nc.scalar.copy(
                            out=xT8[:, hi0:hi0 + 8:2, ts(tt, P)], in_=ps[:]
                        )
                # distance-1 pipeline: burst for chunk ci-1 lands after
                # chunk ci's transposes, hiding the extract latency
                if pending is not None:
                    ptt, pblk = pending
                    mm_burst(wt8, ptt, pblk)
                    if pblk == BLK - 1:
                        neg_mms(ptt)
                    pci = ci - 1
                    if pci == 2:
                        # slab-1 weights early: byte transposes/unpack fill
                        # ACT/DVE slack in the blk0 phase
                        wt_next = unpack_slab(byte_slab(pk_next))
                        pk_next = pk_dma(2) if NSLAB > 2 else None
                    if pci == 5:
                        bias_cur = bias_dma(0)
                pending = (tt, blk)
            ptt, pblk = pending
            mm_burst(wt8, ptt, pblk)
            neg_mms(ptt)
            for tsub in range(TT):
                epilogue(0, tsub, pss[tsub], bias_cur)
            wt_cur = wt_next

            # --- steady slabs (pk for slab sl+1 was issued at slab sl-1) ---
            for sl in range(1, NSLAB):
                wt8 = wt_cur[:].bitcast(FP8)
                bbc = bias_dma(sl)
                pss = [
                    ps_mm.tile([P, O_SLAB], F32, name=f"ps{t}", tag="ps_y")
                    for t in range(TT)
                ]
                for tsub in range(TT):
                    if sl == NSLAB - 1 and tsub == TT - 1:
                        # final tile rides two o-half psum groups with
                        # staggered stops: half the epilogue + store overlaps
                        # the other half's matmuls, trimming the drain tail
                        for half in range(2):
                            ph = ps_mm.tile(
                                [P, O_SLAB // 2], F32, name=f"psh{half}",
                                tag="ps_y",
                            )
                            o0 = half * (O_SLAB // 2)

                            def rhs_half(u):
                                return wt8[:, u, 2 * o0:2 * o0 + O_SLAB] \
                                    .rearrange("p (o s) -> p s o", s=2)

                            for u in range(NU):
                                nc.tensor.matmul(
                                    ph[:], mm_lhs_hi(u, tsub * P, P),
                                    rhs_half(u),
                                    start=(u == 0), stop=False, perf_mode=DR,
                                )
                            for h in range(LO_U):
                                nc.tensor.matmul(
                                    ph[:], mm_lhs_lo(h, tsub * P, P),
                                    rhs_half(h),
                                    start=False, stop=(h == LO_U - 1),
                                    perf_mode=DR,
                                )
                            y_sb = y_pool.tile([P, O_SLAB // 2], F32)
                            nc.vector.scalar_tensor_tensor(
                                out=y_sb[:], in0=ph[:],
                                scalar=neg_s[:, tsub:tsub + 1],
                                in1=bbc[:, o0:o0 + O_SLAB // 2],
                                op0=ADD, op1=ADD,
                            )
                            nc.sync.dma_start(
                                y_d[ts(tsub, P),
                                    sl * O_SLAB + o0:
                                    sl * O_SLAB + o0 + O_SLAB // 2],
                                y_sb[:],
                            )
                        continue
                    for u in range(NU):
                        nc.tensor.matmul(
                            pss[tsub][:], mm_lhs_hi(u, tsub * P, P),
                            mm_rhs(wt8, u),
                            start=(u == 0), stop=False, perf_mode=DR,
                        )
                    for h in range(LO_U):
                        nc.tensor.matmul(
                            pss[tsub][:], mm_lhs_lo(h, tsub * P, P),
                            mm_rhs(wt8, h),
                            start=False, stop=(h == LO_U - 1), perf_mode=DR,
                        )
                    epilogue(sl, tsub, pss[tsub], bbc)
                    if tsub == 1 and pk_next is not None:
                        wt_cur = unpack_slab(byte_slab(pk_next))
                        pk_next = pk_dma(sl + 2) if sl + 2 < NSLAB else None

    nc.compile()
    return nc


_NC = None


def _get_nc():
    global _NC
    if _NC is None:
        _NC = build()
    return _NC


def run(x, packed_weight, bias, trace=False):
    x = np.ascontiguousarray(np.asarray(x, dtype=np.float32))
    pw = np.ascontiguousarray(np.asarray(packed_weight, dtype=np.uint8))
    bias = np.ascontiguousarray(np.asarray(bias, dtype=np.float32))
    assert x.shape == (B_DIM, S_DIM, I_DIM)
    assert pw.shape == (O_DIM, I_DIM // 8)
    assert bias.shape == (O_DIM,)

    nc = _get_nc()
    xs = x.reshape(T_FULL, I_DIM)
    in_maps = [
        {
            "x": np.ascontiguousarray(xs[c * T_SHARD:(c + 1) * T_SHARD]),
            "pw": pw,
            "bias": bias,
        }
        for c in range(N_CORES)
    ]
    res = run_bass_kernel_spmd(nc, in_maps, list(range(N_CORES)), trace=trace)
    y = np.concatenate(
        [res.results[c][OUT_NAME] for c in range(N_CORES)], axis=0
    )
    return y.reshape(B_DIM, S_DIM, O_DIM), res


def kernel(x, packed_weight, bias):
    y, _ = run(x, packed_weight, bias, trace=False)
    return y
